# revision 27
# baseline (speedup 1.0000x reference)
"""Trainium2 Bass kernel for nn_AutoEncIndex_33887291965861 (topk_masking).

Reference computation:
    soft  = softmax((mat + noise) / temperature)            [training w/ gumbel]
    index = top_k(soft, J).indices                          (full descending sort)
    sel   = greedy row-by-row assignment (first J rows pick the best
            still-unused joint; later rows pick their argmax)
    out   = stop_grad(one_hot(sel)) - stop_grad(mat) + mat

Key facts used here:
  * (0 - m) + m == +0.0 exactly in IEEE fp32, so the output is an exact
    one-hot matrix except the selected entry is (1 - m) + m which is within
    1-2 ulp of 1.0.  Emitting exactly 1.0 keeps the total relative error
    at ~2e-7.
  * softmax and /temperature are strictly monotone per row, so the selection
    order is the order of w = mat + noise (fp32), with lowest-index
    tie-breaking (lax.top_k semantics == vector-engine max_index semantics).
  * The greedy pass over the first J rows selects, for row r, the
    still-available joint with the highest w[r] value (proof: the first
    available joint in row r's descending order always sits within the
    first r+1 positions by pigeonhole, which is exactly the cols<=r window
    the reference uses).  Rows >= J just take their argmax.

Device kernel (SPMD over 8 cores, row-sharded, 4096 rows/core; mode "v3"
with out_dt "packi" - the measured-fastest configuration):
  * Input: host interleaves [mat;noise] so every 4 MB chunk (512 rows) is
    one fully-contiguous HBM span, 32 KB contiguous per partition line.
    A single chunk DMA on the sync (SP) HWDGE ring, and that ring carries
    ONLY input loads, so chunks stream back-to-back at the measured
    ~345 GB/s per-core rate (the split mat/noise descriptor layout only
    reaches ~326 GB/s; splitting input across both HWDGE rings reaches
    347 GB/s loads-only but collapses to ~140 us/pass when compute
    coexists, so single-ring contiguous wins).
  * w = mat + noise on gpsimd (Pool), freeing the vector engine.
  * DVE: per-segment argmax via max/max_index, plus one strided batch of
    index math per chunk (byte index bi = idx>>3, bit value 2^(idx&7) as
    f32 bits) feeding the packed one-hot.
  * ACT: bit-packed one-hot (128 B/row): ab = Abs(-iota128 + bi), then
    byte = Relu(ab*(-v) + v) = v at the byte position, 0 elsewhere
    (v = 2^(idx&7), exact in u8).  Output DMA on the ACT HWDGE ring so it
    never stalls the input FIFO.
  Memory bound: 32.5 MB HBM traffic per core per pass (32 MiB in +
  512 KB packed out); steady state ~98 us/pass = within ~2% of the
  measured pure-input-DMA ceiling (~96.6 us).  The vector engine
  (~9.5 us/chunk), Pool (~8.5 us/chunk) and ACT (~3 us/chunk) all sit
  below the 12.2 us/chunk DMA period, fully overlapped.

Host: the inherently-sequential greedy over the first 1024 rows (tiny), then
patch those rows of the gathered output; unpackbits decodes the device's
bit-packed one-hot (exact).
"""

import os

import numpy as np

HW = 32768
J = 1024
N_CORES = 8
ROWS_PER_CORE = HW // N_CORES  # 4096
P = 128  # SBUF partitions

_NC_CACHE = {}


def _build_nc(rows_per_core: int, j: int, r: int, onehot_engine: str = "act",
              repeat: int = 1, mode: str = "full", bufs: int = 2,
              out_engine: str = "sync", out_dt: str = "u8", mn_bufs: int = 0):
    """Build the per-core Bass module.

    Input "mn" is [2, rows_per_core, j] fp32 — mat stacked with noise (one
    tensor so each chunk loads with a single DMA instruction / single
    semaphore: TRN2 compute instructions can carry only one sync wait).
    Output "out" is the exact one-hot of the per-row argmax of mat + noise.
    r = rows per partition per chunk (chunk covers 128*r rows).
    """
    import concourse.bacc as bacc
    import concourse.mybir as mybir
    from concourse.tile import TileContext

    chunk_rows = P * r
    assert rows_per_core % chunk_rows == 0, (rows_per_core, chunk_rows)
    n_chunks = rows_per_core // chunk_rows
    f32 = mybir.dt.float32

    # Bacc (not raw Bass): its finalize() runs generate_event_semaphores,
    # which splits multi-sem waits — TRN2 instructions carry at most one.
    nc = bacc.Bacc()
    if mode in ("v3", "v4", "v4a"):
        return _build_nc_v3(nc, mybir, TileContext, rows_per_core, j, r, repeat,
                            bufs, out_dt, mn_bufs, mode)
    if mode.startswith("v5"):
        return _build_nc_v5(nc, mybir, TileContext, rows_per_core, j, r, repeat,
                            bufs, out_dt, mn_bufs, mode)
    pack = out_dt == "pack"
    odt = {"f32": f32, "u8": mybir.dt.uint8, "bf16": mybir.dt.bfloat16,
           "pack": mybir.dt.uint8}[out_dt]
    # packed mode: 1024 one-hot bits -> 128 bytes per row (byte idx>>3 holds
    # 1 << (idx & 7)); host unpacks with np.unpackbits(bitorder="little")
    jo = j // 8 if pack else j
    ilv = mode == "ilv"
    if ilv:
        # host pre-interleaves so every chunk DMA reads one fully-contiguous
        # 4 MB block (single HBM stream instead of mat/noise 16 MB apart)
        mn = nc.dram_tensor(
            "mn", [rows_per_core // (P * r), P, 2, r * j], f32,
            kind="ExternalInput")
        mnv = mn[:, :, :, :]
    else:
        mn = nc.dram_tensor("mn", [2, rows_per_core, j], f32, kind="ExternalInput")
        # chunk c, partition p holds rows (c*128 + p)*r .. +r-1
        mnv = mn[:, :, :].rearrange("t (c p r) m -> c p t (r m)", p=P, r=r)
    out = nc.dram_tensor("out", [rows_per_core, jo], odt, kind="ExternalOutput")
    outv = out[:, :].rearrange("(c p r) m -> c p (r m)", p=P, r=r)

    out_dma = {"sync": nc.sync, "scalar": nc.scalar, "gpsimd": nc.gpsimd}[out_engine]
    with TileContext(nc) as tc:
        with (
            tc.tile_pool(name="const", bufs=1) as cpool,
            tc.tile_pool(name="work", bufs=bufs) as pool,
            tc.tile_pool(name="mnp", bufs=mn_bufs or bufs) as mnpool,
            tc.tile_pool(name="small", bufs=3) as spool,
        ):
            iota_i = cpool.tile([P, j], mybir.dt.int32)
            nc.gpsimd.iota(iota_i[:], [[1, j]], channel_multiplier=0)
            iota_f = cpool.tile([P, j], f32)
            nc.vector.tensor_copy(iota_f[:], iota_i[:])

            for c in [c for _ in range(repeat) for c in range(n_chunks)]:
                tmn = mnpool.tile([P, 2, r * j], f32, tag="mn")
                if mode in ("split2", "loadonly2"):
                    # mat half on the SP HWDGE ring, noise half on the ACT ring
                    nc.sync.dma_start(tmn[:, 0, :], mnv[c][:, 0, :])
                    nc.scalar.dma_start(tmn[:, 1, :], mnv[c][:, 1, :])
                else:
                    nc.sync.dma_start(tmn[:, :, :], mnv[c])
                if mode in ("loadonly", "loadonly2"):
                    continue
                if mode == "dmaonly":
                    ot = pool.tile([P, r * j], odt, tag="out")
                    nc.vector.tensor_copy(ot[:], tmn[:, 0, :])
                    out_dma.dma_start(outv[c], ot[:])
                    continue
                w = pool.tile([P, r * j], f32, tag="w")
                nc.vector.tensor_add(w[:], tmn[:, 0, :], tmn[:, 1, :])
                ot = pool.tile([P, r * jo], odt, tag="out")
                mx = spool.tile([P, 8 * r], f32, tag="mx")
                ix = spool.tile([P, 8 * r], mybir.dt.uint32, tag="ix")
                for s in range(r):
                    seg = w[:, s * j : (s + 1) * j]
                    oseg = ot[:, s * jo : (s + 1) * jo]
                    nc.vector.max(mx[:, 8 * s : 8 * s + 8], seg)
                    nc.vector.max_index(ix[:, 8 * s : 8 * s + 8], mx[:, 8 * s : 8 * s + 8], seg)
                    if pack:
                        ixs = ix[:, 8 * s : 8 * s + 1]
                        bi = spool.tile([P, 1], mybir.dt.uint32, tag="bi")
                        nc.vector.tensor_scalar(
                            bi[:], ixs, 3, None, op0=mybir.AluOpType.logical_shift_right)
                        rem = spool.tile([P, 1], mybir.dt.uint32, tag="rem")
                        nc.vector.tensor_scalar(
                            rem[:], ixs, 7, None, op0=mybir.AluOpType.bitwise_and)
                        # v = 2^rem exactly: f32 bit pattern (rem+127) << 23
                        vb = spool.tile([P, 1], mybir.dt.uint32, tag="vb")
                        nc.vector.tensor_scalar(
                            vb[:], rem[:], 127, None, op0=mybir.AluOpType.add)
                        ve = spool.tile([P, 1], mybir.dt.uint32, tag="ve")
                        nc.vector.tensor_scalar(
                            ve[:], vb[:], 23, None,
                            op0=mybir.AluOpType.logical_shift_left)
                        bf = spool.tile([P, 1], f32, tag="bf")
                        nc.vector.tensor_scalar_mul(bf[:], bi[:], 1.0)
                        nc.vector.tensor_scalar(
                            oseg, iota_f[:, :jo], bf[:], ve[:].bitcast(f32),
                            op0=mybir.AluOpType.is_equal,
                            op1=mybir.AluOpType.mult)
                    elif onehot_engine == "act":
                        # one-hot on the scalar engine: Relu(1 - |iota - idx|)
                        ixn = spool.tile([P, 1], f32, tag="ixn")
                        nc.vector.tensor_scalar_mul(ixn[:], ix[:, 8 * s : 8 * s + 1], -1.0)
                        ab = spool.tile([P, j], f32, tag="abs")
                        nc.scalar.activation(
                            ab[:], iota_f[:], mybir.ActivationFunctionType.Abs,
                            bias=ixn[:], scale=1.0,
                        )
                        nc.scalar.activation(
                            oseg, ab[:], mybir.ActivationFunctionType.Relu,
                            bias=1.0, scale=-1.0,
                        )
                    else:
                        # one-hot on the vector engine: (iota == idx), f32 compare
                        ixf = spool.tile([P, 1], f32, tag="ixf")
                        nc.vector.tensor_scalar_mul(ixf[:], ix[:, 8 * s : 8 * s + 1], 1.0)
                        nc.vector.tensor_scalar(
                            oseg, iota_f[:], ixf[:], None,
                            op0=mybir.AluOpType.is_equal,
                        )
                out_dma.dma_start(outv[c], ot[:])
    nc.finalize()
    return nc


def _build_nc_v3(nc, mybir, TileContext, rows_per_core, j, r, repeat, bufs,
                 out_dt, mn_bufs, mode="v3"):
    """v3/v4: engine-balanced, DMA-roofline layout.

    - v3: one 4 MB chunk DMA on the sync (SP) HWDGE ring; output on scalar.
    - v4: mat half on sync ring + noise half on scalar ring (two HWDGE
      queues stream concurrently: HW measures 347 GB/s vs 326 single-ring);
      output via gpsimd SWDGE so it never blocks either input FIFO.
    - v4a: v4 input split but output on the scalar ring (FIFO-risk A/B).
    - gpsimd (Pool) does w = mat + noise, freeing DVE.
    - DVE does max / max_index per row segment plus one strided batch of
      index math per chunk (byte index + bit value for the packed one-hot).
    - ACT builds the bit-packed one-hot (128 B/row) as two 128-elem
      activations per segment.
    Output: packed one-hot bits, u8 [rows, j//8] (host unpackbits), so HBM
    write traffic is j/8 bytes/row instead of j.
    """
    P = 128
    f32 = mybir.dt.float32
    u32 = mybir.dt.uint32
    chunk_rows = P * r
    assert rows_per_core % chunk_rows == 0
    n_chunks = rows_per_core // chunk_rows
    ilv = out_dt == "packi"
    jo = j // 8

    if ilv:
        # host pre-interleaves: each chunk is one contiguous 4 MB HBM span,
        # 32 KB contiguous per partition line
        mn = nc.dram_tensor("mn", [n_chunks, P, 2, r * j], f32,
                            kind="ExternalInput")
        mnv = mn[:, :, :, :]
    else:
        mn = nc.dram_tensor("mn", [2, rows_per_core, j], f32, kind="ExternalInput")
        mnv = mn[:, :, :].rearrange("t (c p r) m -> c p t (r m)", p=P, r=r)
    out = nc.dram_tensor("out", [rows_per_core, jo], mybir.dt.uint8,
                         kind="ExternalOutput")
    outv = out[:, :].rearrange("(c p r) m -> c p (r m)", p=P, r=r)

    with TileContext(nc) as tc:
        with (
            tc.tile_pool(name="const", bufs=1) as cpool,
            tc.tile_pool(name="work", bufs=bufs) as pool,
            tc.tile_pool(name="mnp", bufs=mn_bufs or bufs) as mnpool,
            tc.tile_pool(name="small", bufs=3) as spool,
        ):
            iota_i = cpool.tile([P, jo], mybir.dt.int32)
            nc.gpsimd.iota(iota_i[:], [[1, jo]], channel_multiplier=0)
            iota_f = cpool.tile([P, jo], f32)
            nc.vector.tensor_copy(iota_f[:], iota_i[:])

            split_in = mode in ("v4", "v4a")
            out_dma = nc.gpsimd if mode == "v4" else nc.scalar
            for c in [c for _ in range(repeat) for c in range(n_chunks)]:
                tmn = mnpool.tile([P, 2, r * j], f32, tag="mn")
                if split_in:
                    if ilv:
                        nc.sync.dma_start(tmn[:, 0, :], mnv[c, :, 0, :])
                        nc.scalar.dma_start(tmn[:, 1, :], mnv[c, :, 1, :])
                    else:
                        nc.sync.dma_start(tmn[:, 0, :], mnv[c][:, 0, :])
                        nc.scalar.dma_start(tmn[:, 1, :], mnv[c][:, 1, :])
                elif ilv:
                    nc.sync.dma_start(tmn[:, :, :], mnv[c, :, :, :])
                else:
                    nc.sync.dma_start(tmn[:, :, :], mnv[c])
                w = pool.tile([P, r * j], f32, tag="w")
                nc.gpsimd.tensor_add(w[:], tmn[:, 0, :], tmn[:, 1, :])
                ot = pool.tile([P, r * jo], mybir.dt.uint8, tag="out")
                mx = spool.tile([P, 8 * r], f32, tag="mx")
                ix = spool.tile([P, 8, r], u32, tag="ix")
                for s in range(r):
                    seg = w[:, s * j : (s + 1) * j]
                    nc.vector.max(mx[:, 8 * s : 8 * s + 8], seg)
                    nc.vector.max_index(ix[:, :, s], mx[:, 8 * s : 8 * s + 8], seg)
                # batch index math over the r argmax heads (strided [P, r])
                heads = ix[:, 0, :]  # [P, r] u32, stride 8 elems
                bi = spool.tile([P, r], u32, tag="bi")
                nc.vector.tensor_scalar(
                    bi[:], heads, 3, None, op0=mybir.AluOpType.logical_shift_right)
                bif = spool.tile([P, r], f32, tag="bif")
                nc.vector.tensor_scalar_mul(bif[:], bi[:], 1.0)
                rem = spool.tile([P, r], u32, tag="rem")
                nc.vector.tensor_scalar(
                    rem[:], heads, 7, None, op0=mybir.AluOpType.bitwise_and)
                vb = spool.tile([P, r], u32, tag="vb")
                nc.vector.tensor_scalar(
                    vb[:], rem[:], 127, None, op0=mybir.AluOpType.add)
                ve = spool.tile([P, r], u32, tag="ve")
                nc.vector.tensor_scalar(
                    ve[:], vb[:], 23, None, op0=mybir.AluOpType.logical_shift_left)
                nv = spool.tile([P, r], f32, tag="nv")
                nc.vector.tensor_scalar_mul(nv[:], ve[:].bitcast(f32), -1.0)
                for s in range(r):
                    oseg = ot[:, s * jo : (s + 1) * jo]
                    ab = spool.tile([P, jo], f32, tag="abs")
                    nc.scalar.activation(
                        ab[:], iota_f[:], mybir.ActivationFunctionType.Abs,
                        bias=bif[:, s : s + 1], scale=-1.0)
                    nc.scalar.activation(
                        oseg, ab[:], mybir.ActivationFunctionType.Relu,
                        bias=ve[:, s : s + 1].bitcast(f32),
                        scale=nv[:, s : s + 1])
                out_dma.dma_start(outv[c], ot[:])
    nc.finalize()
    return nc


def _build_nc_v5(nc, mybir, TileContext, rows_per_core, j, r, repeat, bufs,
                 out_dt, mn_bufs, mode="v5"):
    """v5: two-ring input streaming at full rate.

    - mat half of each chunk on the sync (SP) HWDGE ring, noise half on the
      scalar (ACT) ring.  ACT runs NO compute at all, so its ring feeds
      noise chunks back-to-back (the v4 lesson: activations in the ACT
      stream FIFO-block the next noise load).
    - output DMA rides the sync ring DELAYED BY TWO CHUNKS, so its data is
      always already computed when the trigger issues - no FIFO stall.
    - gpsimd (Pool): w = mat + noise.  DVE: max/max_index per segment,
      batched index math, and the bit-packed one-hot bytes via
      (iota == byte_idx) * 2^bit with per-partition scalar operands.
    - v5l: loads only (input-ceiling measurement variant).
    """
    P = 128
    f32 = mybir.dt.float32
    u32 = mybir.dt.uint32
    chunk_rows = P * r
    assert rows_per_core % chunk_rows == 0
    n_chunks = rows_per_core // chunk_rows
    ilv = out_dt == "packi"
    loadonly = mode == "v5l"
    no_out = mode == "v5no"     # loads + add + DVE, no output DMA
    no_add = mode == "v5na"     # loads + DVE on mat only + out, no Pool add
    one_ring = mode == "v5s2"   # split DMAs but both on sync; out on scalar
    two_tiles = mode == "v5t2"  # mat and noise in separate tiles
    jo = j // 8

    if ilv:
        mn = nc.dram_tensor("mn", [n_chunks, P, 2, r * j], f32,
                            kind="ExternalInput")
    else:
        mn = nc.dram_tensor("mn", [2, rows_per_core, j], f32, kind="ExternalInput")
        mnv = mn[:, :, :].rearrange("t (c p r) m -> c p t (r m)", p=P, r=r)
    out = nc.dram_tensor("out", [rows_per_core, jo], mybir.dt.uint8,
                         kind="ExternalOutput")
    outv = out[:, :].rearrange("(c p r) m -> c p (r m)", p=P, r=r)

    with TileContext(nc) as tc:
        with (
            tc.tile_pool(name="const", bufs=1) as cpool,
            tc.tile_pool(name="work", bufs=bufs) as pool,
            tc.tile_pool(name="mnp", bufs=mn_bufs or bufs) as mnpool,
            tc.tile_pool(name="otp", bufs=4) as opool,
            tc.tile_pool(name="small", bufs=3) as spool,
        ):
            iota_i = cpool.tile([P, jo], mybir.dt.int32)
            nc.gpsimd.iota(iota_i[:], [[1, jo]], channel_multiplier=0)
            iota_f = cpool.tile([P, jo], f32)
            nc.vector.tensor_copy(iota_f[:], iota_i[:])

            pending = []

            def flush_one():
                cc, ot_cc = pending.pop(0)
                (nc.scalar if mode == "v5s2" else nc.sync).dma_start(
                    outv[cc], ot_cc[:])

            for c in [c for _ in range(repeat) for c in range(n_chunks)]:
                noise_q = nc.sync if one_ring else nc.scalar
                if two_tiles:
                    tmat = mnpool.tile([P, r * j], f32, tag="tmat")
                    tnoi = mnpool.tile([P, r * j], f32, tag="tnoi")
                    srcm = mn[c, :, 0, :] if ilv else mnv[c][:, 0, :]
                    srcn = mn[c, :, 1, :] if ilv else mnv[c][:, 1, :]
                    nc.sync.dma_start(tmat[:], srcm)
                    noise_q.dma_start(tnoi[:], srcn)
                    t0, t1 = tmat[:], tnoi[:]
                else:
                    tmn = mnpool.tile([P, 2, r * j], f32, tag="mn")
                    if ilv:
                        nc.sync.dma_start(tmn[:, 0, :], mn[c, :, 0, :])
                        noise_q.dma_start(tmn[:, 1, :], mn[c, :, 1, :])
                    else:
                        nc.sync.dma_start(tmn[:, 0, :], mnv[c][:, 0, :])
                        noise_q.dma_start(tmn[:, 1, :], mnv[c][:, 1, :])
                    t0, t1 = tmn[:, 0, :], tmn[:, 1, :]
                if loadonly:
                    continue
                if no_add:
                    w_ap = t0
                else:
                    w = pool.tile([P, r * j], f32, tag="w")
                    nc.gpsimd.tensor_add(w[:], t0, t1)
                    w_ap = w[:]
                ot = opool.tile([P, r * jo], mybir.dt.uint8, tag="out")
                mx = spool.tile([P, 8 * r], f32, tag="mx")
                ix = spool.tile([P, 8, r], u32, tag="ix")
                for s in range(r):
                    seg = w_ap[:, s * j : (s + 1) * j]
                    nc.vector.max(mx[:, 8 * s : 8 * s + 8], seg)
                    nc.vector.max_index(ix[:, :, s], mx[:, 8 * s : 8 * s + 8], seg)
                heads = ix[:, 0, :]  # [P, r] u32, stride 8 elems
                bi = spool.tile([P, r], u32, tag="bi")
                nc.vector.tensor_scalar(
                    bi[:], heads, 3, None, op0=mybir.AluOpType.logical_shift_right)
                bif = spool.tile([P, r], f32, tag="bif")
                nc.vector.tensor_scalar_mul(bif[:], bi[:], 1.0)
                rem = spool.tile([P, r], u32, tag="rem")
                nc.vector.tensor_scalar(
                    rem[:], heads, 7, None, op0=mybir.AluOpType.bitwise_and)
                vb = spool.tile([P, r], u32, tag="vb")
                nc.vector.tensor_scalar(
                    vb[:], rem[:], 127, None, op0=mybir.AluOpType.add)
                ve = spool.tile([P, r], u32, tag="ve")
                nc.vector.tensor_scalar(
                    ve[:], vb[:], 23, None, op0=mybir.AluOpType.logical_shift_left)
                for s in range(r):
                    oseg = ot[:, s * jo : (s + 1) * jo]
                    nc.vector.tensor_scalar(
                        oseg, iota_f[:], bif[:, s : s + 1],
                        ve[:, s : s + 1].bitcast(f32),
                        op0=mybir.AluOpType.is_equal,
                        op1=mybir.AluOpType.mult)
                if no_out:
                    continue
                pending.append((c, ot))
                if len(pending) > 2:
                    if one_ring:
                        cc, ot_cc = pending.pop(0)
                        nc.scalar.dma_start(outv[cc], ot_cc[:])
                    else:
                        flush_one()
            while pending:
                flush_one()
    nc.finalize()
    return nc


def _get_nc(rows_per_core=ROWS_PER_CORE, j=J, r=4, onehot_engine=None, repeat=1,
            mode=None, bufs=2, out_engine="scalar", out_dt=None, mn_bufs=0):
    if mode is None:
        mode = os.environ.get("KERNEL_MODE", "v3")
    if onehot_engine is None:
        onehot_engine = os.environ.get("KERNEL_ONEHOT", "act")
    if out_dt is None:
        out_dt = os.environ.get("KERNEL_OUT_DT", "packi")
    key = (rows_per_core, j, r, onehot_engine, repeat, mode, bufs, out_engine, out_dt,
           mn_bufs)
    if key not in _NC_CACHE:
        _NC_CACHE[key] = _build_nc(*key)
    return _NC_CACHE[key]


def _greedy_select(w_first: np.ndarray) -> np.ndarray:
    """Sequential greedy: row r takes the available joint with max w[r].

    Equivalent to the reference's scan over descending top-k indices.
    """
    n = w_first.shape[0]
    avail = np.ones(n, dtype=bool)
    sel = np.empty(n, dtype=np.int64)
    neg_inf = np.float32(-np.inf)
    for r in range(n):
        row = np.where(avail, w_first[r], neg_inf)
        s = int(np.argmax(row))
        sel[r] = s
        avail[s] = False
    return sel


_RUNNER_CACHE = {}


def _make_runner(r: int = 4, onehot_engine=None, repeat: int = 1, mode: str = None,
                 bufs: int = 2, out_engine: str = "scalar", out_dt=None, mn_bufs: int = 0):
    """Cached runner around run_bass_kernel_spmd.

    The first call goes through run_bass_kernel_spmd (the supported axon/PJRT
    path); during it we capture the jitted SPMD callable that
    run_bass_via_pjrt builds internally, so subsequent calls (and timing
    loops) reuse the compiled executable instead of re-tracing/re-compiling
    (run_bass_via_pjrt creates a fresh jit closure per invocation).
    """
    if mode is None:
        mode = os.environ.get("KERNEL_MODE", "v3")
    if out_dt is None:
        out_dt = os.environ.get("KERNEL_OUT_DT", "packi")
    key = (r, onehot_engine, repeat, mode, bufs, out_engine, out_dt, mn_bufs)
    if key in _RUNNER_CACHE:
        return _RUNNER_CACHE[key]

    import jax
    from concourse.bass_utils import run_bass_kernel_spmd

    nc = _get_nc(ROWS_PER_CORE, J, r, onehot_engine, repeat, mode, bufs, out_engine,
                 out_dt, mn_bufs)
    state = {"fn": None}

    def runner(mn_global: np.ndarray) -> np.ndarray:
        """mn_global: (2*N_CORES, ROWS_PER_CORE, J) per-core [mat, noise]
        pairs. Returns (HW, J) output."""
        if state["fn"] is None:
            per = mn_global.shape[0] // N_CORES
            in_maps = [{"mn": mn_global[per * k : per * (k + 1)]} for k in range(N_CORES)]
            orig_jit = jax.jit

            def capturing_jit(f, *a, **kw):
                g = orig_jit(f, *a, **kw)
                if "donate_argnums" in kw and kw.get("keep_unused"):
                    state["fn"] = g
                return g

            jax.jit = capturing_jit
            try:
                res = run_bass_kernel_spmd(nc, in_maps, core_ids=list(range(N_CORES)))
            finally:
                jax.jit = orig_jit
            out = np.concatenate([r_["out"] for r_ in res.results], axis=0)
            state["out_np_dtype"] = out.dtype
            state["out_shape"] = out.shape
            return out
        outs = state["fn"](mn_global, np.zeros(state["out_shape"], state["out_np_dtype"]))
        out = outs[0] if isinstance(outs, (tuple, list)) else outs
        return np.asarray(out)

    runner.state = state
    runner.stack = ((lambda m, n: stack_inputs_ilv(m, n, r))
                    if out_dt == "packi" else stack_inputs)
    _RUNNER_CACHE[key] = runner
    return runner


def stack_inputs(mat: np.ndarray, noise: np.ndarray) -> np.ndarray:
    """Global (2*N_CORES, ROWS_PER_CORE, J): per-core [mat_shard, noise_shard]
    pairs along axis 0, so a P("core") shard is exactly the NEFF's (2, rows, J)
    "mn" tensor."""
    m3 = mat.reshape(N_CORES, ROWS_PER_CORE, J)
    n3 = noise.reshape(N_CORES, ROWS_PER_CORE, J)
    return np.stack([m3, n3], axis=1).reshape(2 * N_CORES, ROWS_PER_CORE, J)


def stack_inputs_ilv(mat: np.ndarray, noise: np.ndarray, r: int = 4) -> np.ndarray:
    """Interleaved layout: global (N_CORES*n_chunks, P, 2, r*J); every chunk is
    one contiguous 4 MB block on device."""
    nck = ROWS_PER_CORE // (P * r)
    m5 = mat.reshape(N_CORES * nck, P, r * J)
    n5 = noise.reshape(N_CORES * nck, P, r * J)
    return np.ascontiguousarray(np.stack([m5, n5], axis=2))


def run_device(mat: np.ndarray, noise: np.ndarray, r: int = 4, onehot_engine=None):
    """Shard row-wise over 8 cores, run the Bass kernel, gather."""
    runner = _make_runner(r, onehot_engine)
    out = runner(runner.stack(mat, noise))
    return np.asarray(out)


def kernel(sgt_trans_mat, gumbel_noise, use_gumbel_noise=1, is_training=1,
           temperature=30):
    mat = np.ascontiguousarray(np.asarray(sgt_trans_mat, dtype=np.float32))
    assert mat.shape == (HW, J), mat.shape
    training = bool(int(np.asarray(is_training)))
    use_g = training and bool(int(np.asarray(use_gumbel_noise)))
    if use_g:
        noise = np.ascontiguousarray(np.asarray(gumbel_noise, dtype=np.float32))
    else:
        # selection order falls back to mat itself; temperature never matters
        noise = np.zeros_like(mat)

    out = run_device(mat, noise)
    # device output may be bit-packed/uint8/bf16 (exact for one-hot); f32 it
    if out.shape[1] == J // 8:
        out = np.unpackbits(np.ascontiguousarray(out), axis=1,
                            bitorder="little").astype(np.float32)
    elif out.dtype != np.float32:
        out = out.astype(np.float32)
    elif not out.flags.writeable:
        out = out.copy()

    # Host-side greedy over the first J rows (inherently sequential, tiny),
    # then patch those rows of the output.
    w_first = mat[:J] + noise[:J]  # same IEEE fp32 add as the device
    sel = _greedy_select(w_first)
    out[:J] = 0.0
    out[np.arange(J), sel] = np.float32(1.0)
    return out



# revision 34
# speedup vs baseline: 1.0394x; 1.0394x over previous
"""Trainium2 Bass kernel for nn_AutoEncIndex_33887291965861 (topk_masking).

Reference computation:
    soft  = softmax((mat + noise) / temperature)            [training w/ gumbel]
    index = top_k(soft, J).indices                          (full descending sort)
    sel   = greedy row-by-row assignment (first J rows pick the best
            still-unused joint; later rows pick their argmax)
    out   = stop_grad(one_hot(sel)) - stop_grad(mat) + mat

Key facts used here:
  * (0 - m) + m == +0.0 exactly in IEEE fp32, so the output is an exact
    one-hot matrix except the selected entry is (1 - m) + m which is within
    1-2 ulp of 1.0.  Emitting exactly 1.0 keeps the total relative error
    at ~2e-7.
  * softmax and /temperature are strictly monotone per row, so the selection
    order is the order of w = mat + noise (fp32), with lowest-index
    tie-breaking (lax.top_k semantics == vector-engine max_index semantics).
  * The greedy pass over the first J rows selects, for row r, the
    still-available joint with the highest w[r] value (proof: the first
    available joint in row r's descending order always sits within the
    first r+1 positions by pigeonhole, which is exactly the cols<=r window
    the reference uses).  Rows >= J just take their argmax.

Device kernel (SPMD over 8 cores, row-sharded, 4096 rows/core; mode "v3"
with out_dt "packi" - the measured-fastest configuration):
  * Input: host interleaves [mat;noise] so every 4 MB chunk (512 rows) is
    one fully-contiguous HBM span, 32 KB contiguous per partition line.
    A single chunk DMA on the sync (SP) HWDGE ring, and that ring carries
    ONLY input loads, so chunks stream back-to-back at the measured
    ~345 GB/s per-core rate (the split mat/noise descriptor layout only
    reaches ~326 GB/s; splitting input across both HWDGE rings reaches
    347 GB/s loads-only but collapses to ~140 us/pass when compute
    coexists, so single-ring contiguous wins).
  * w = mat + noise on gpsimd (Pool), freeing the vector engine.
  * DVE: per-segment argmax via max/max_index, plus one strided batch of
    index math per chunk (byte index bi = idx>>3, bit value 2^(idx&7) as
    f32 bits) feeding the packed one-hot.
  * ACT: bit-packed one-hot (128 B/row): ab = Abs(-iota128 + bi), then
    byte = Relu(ab*(-v) + v) = v at the byte position, 0 elsewhere
    (v = 2^(idx&7), exact in u8).  Output DMA on the ACT HWDGE ring so it
    never stalls the input FIFO.
  Memory bound: 32.5 MB HBM traffic per core per pass (32 MiB in +
  512 KB packed out); steady state ~98 us/pass = within ~2% of the
  measured pure-input-DMA ceiling (~96.6 us).  The vector engine
  (~9.5 us/chunk), Pool (~8.5 us/chunk) and ACT (~3 us/chunk) all sit
  below the 12.2 us/chunk DMA period, fully overlapped.

Host: the inherently-sequential greedy over the first 1024 rows (tiny), then
patch those rows of the gathered output; unpackbits decodes the device's
bit-packed one-hot (exact).
"""

import os

import numpy as np

HW = 32768
J = 1024
N_CORES = 8
ROWS_PER_CORE = HW // N_CORES  # 4096
P = 128  # SBUF partitions

_NC_CACHE = {}


def _build_nc(rows_per_core: int, j: int, r: int, onehot_engine: str = "act",
              repeat: int = 1, mode: str = "full", bufs: int = 2,
              out_engine: str = "sync", out_dt: str = "u8", mn_bufs: int = 0):
    """Build the per-core Bass module.

    Input "mn" is [2, rows_per_core, j] fp32 — mat stacked with noise (one
    tensor so each chunk loads with a single DMA instruction / single
    semaphore: TRN2 compute instructions can carry only one sync wait).
    Output "out" is the exact one-hot of the per-row argmax of mat + noise.
    r = rows per partition per chunk (chunk covers 128*r rows).
    """
    import concourse.bacc as bacc
    import concourse.mybir as mybir
    from concourse.tile import TileContext

    chunk_rows = P * r
    assert rows_per_core % chunk_rows == 0, (rows_per_core, chunk_rows)
    n_chunks = rows_per_core // chunk_rows
    f32 = mybir.dt.float32

    # Bacc (not raw Bass): its finalize() runs generate_event_semaphores,
    # which splits multi-sem waits — TRN2 instructions carry at most one.
    nc = bacc.Bacc()
    if mode in ("v3", "v3l", "v4", "v4a", "v6"):
        return _build_nc_v3(nc, mybir, TileContext, rows_per_core, j, r, repeat,
                            bufs, out_dt, mn_bufs, mode)
    if mode.startswith("v5"):
        return _build_nc_v5(nc, mybir, TileContext, rows_per_core, j, r, repeat,
                            bufs, out_dt, mn_bufs, mode)
    pack = out_dt == "pack"
    odt = {"f32": f32, "u8": mybir.dt.uint8, "bf16": mybir.dt.bfloat16,
           "pack": mybir.dt.uint8}[out_dt]
    # packed mode: 1024 one-hot bits -> 128 bytes per row (byte idx>>3 holds
    # 1 << (idx & 7)); host unpacks with np.unpackbits(bitorder="little")
    jo = j // 8 if pack else j
    ilv = mode == "ilv"
    if ilv:
        # host pre-interleaves so every chunk DMA reads one fully-contiguous
        # 4 MB block (single HBM stream instead of mat/noise 16 MB apart)
        mn = nc.dram_tensor(
            "mn", [rows_per_core // (P * r), P, 2, r * j], f32,
            kind="ExternalInput")
        mnv = mn[:, :, :, :]
    else:
        mn = nc.dram_tensor("mn", [2, rows_per_core, j], f32, kind="ExternalInput")
        # chunk c, partition p holds rows (c*128 + p)*r .. +r-1
        mnv = mn[:, :, :].rearrange("t (c p r) m -> c p t (r m)", p=P, r=r)
    out = nc.dram_tensor("out", [rows_per_core, jo], odt, kind="ExternalOutput")
    outv = out[:, :].rearrange("(c p r) m -> c p (r m)", p=P, r=r)

    out_dma = {"sync": nc.sync, "scalar": nc.scalar, "gpsimd": nc.gpsimd}[out_engine]
    with TileContext(nc) as tc:
        with (
            tc.tile_pool(name="const", bufs=1) as cpool,
            tc.tile_pool(name="work", bufs=bufs) as pool,
            tc.tile_pool(name="mnp", bufs=mn_bufs or bufs) as mnpool,
            tc.tile_pool(name="small", bufs=3) as spool,
        ):
            iota_i = cpool.tile([P, j], mybir.dt.int32)
            nc.gpsimd.iota(iota_i[:], [[1, j]], channel_multiplier=0)
            iota_f = cpool.tile([P, j], f32)
            nc.vector.tensor_copy(iota_f[:], iota_i[:])

            for c in [c for _ in range(repeat) for c in range(n_chunks)]:
                tmn = mnpool.tile([P, 2, r * j], f32, tag="mn")
                if mode in ("split2", "loadonly2"):
                    # mat half on the SP HWDGE ring, noise half on the ACT ring
                    nc.sync.dma_start(tmn[:, 0, :], mnv[c][:, 0, :])
                    nc.scalar.dma_start(tmn[:, 1, :], mnv[c][:, 1, :])
                else:
                    nc.sync.dma_start(tmn[:, :, :], mnv[c])
                if mode in ("loadonly", "loadonly2"):
                    continue
                if mode == "dmaonly":
                    ot = pool.tile([P, r * j], odt, tag="out")
                    nc.vector.tensor_copy(ot[:], tmn[:, 0, :])
                    out_dma.dma_start(outv[c], ot[:])
                    continue
                w = pool.tile([P, r * j], f32, tag="w")
                nc.vector.tensor_add(w[:], tmn[:, 0, :], tmn[:, 1, :])
                ot = pool.tile([P, r * jo], odt, tag="out")
                mx = spool.tile([P, 8 * r], f32, tag="mx")
                ix = spool.tile([P, 8 * r], mybir.dt.uint32, tag="ix")
                for s in range(r):
                    seg = w[:, s * j : (s + 1) * j]
                    oseg = ot[:, s * jo : (s + 1) * jo]
                    nc.vector.max(mx[:, 8 * s : 8 * s + 8], seg)
                    nc.vector.max_index(ix[:, 8 * s : 8 * s + 8], mx[:, 8 * s : 8 * s + 8], seg)
                    if pack:
                        ixs = ix[:, 8 * s : 8 * s + 1]
                        bi = spool.tile([P, 1], mybir.dt.uint32, tag="bi")
                        nc.vector.tensor_scalar(
                            bi[:], ixs, 3, None, op0=mybir.AluOpType.logical_shift_right)
                        rem = spool.tile([P, 1], mybir.dt.uint32, tag="rem")
                        nc.vector.tensor_scalar(
                            rem[:], ixs, 7, None, op0=mybir.AluOpType.bitwise_and)
                        # v = 2^rem exactly: f32 bit pattern (rem+127) << 23
                        vb = spool.tile([P, 1], mybir.dt.uint32, tag="vb")
                        nc.vector.tensor_scalar(
                            vb[:], rem[:], 127, None, op0=mybir.AluOpType.add)
                        ve = spool.tile([P, 1], mybir.dt.uint32, tag="ve")
                        nc.vector.tensor_scalar(
                            ve[:], vb[:], 23, None,
                            op0=mybir.AluOpType.logical_shift_left)
                        bf = spool.tile([P, 1], f32, tag="bf")
                        nc.vector.tensor_scalar_mul(bf[:], bi[:], 1.0)
                        nc.vector.tensor_scalar(
                            oseg, iota_f[:, :jo], bf[:], ve[:].bitcast(f32),
                            op0=mybir.AluOpType.is_equal,
                            op1=mybir.AluOpType.mult)
                    elif onehot_engine == "act":
                        # one-hot on the scalar engine: Relu(1 - |iota - idx|)
                        ixn = spool.tile([P, 1], f32, tag="ixn")
                        nc.vector.tensor_scalar_mul(ixn[:], ix[:, 8 * s : 8 * s + 1], -1.0)
                        ab = spool.tile([P, j], f32, tag="abs")
                        nc.scalar.activation(
                            ab[:], iota_f[:], mybir.ActivationFunctionType.Abs,
                            bias=ixn[:], scale=1.0,
                        )
                        nc.scalar.activation(
                            oseg, ab[:], mybir.ActivationFunctionType.Relu,
                            bias=1.0, scale=-1.0,
                        )
                    else:
                        # one-hot on the vector engine: (iota == idx), f32 compare
                        ixf = spool.tile([P, 1], f32, tag="ixf")
                        nc.vector.tensor_scalar_mul(ixf[:], ix[:, 8 * s : 8 * s + 1], 1.0)
                        nc.vector.tensor_scalar(
                            oseg, iota_f[:], ixf[:], None,
                            op0=mybir.AluOpType.is_equal,
                        )
                out_dma.dma_start(outv[c], ot[:])
    nc.finalize()
    return nc


def _build_nc_v3(nc, mybir, TileContext, rows_per_core, j, r, repeat, bufs,
                 out_dt, mn_bufs, mode="v3"):
    """v3/v4: engine-balanced, DMA-roofline layout.

    - v3: one 4 MB chunk DMA on the sync (SP) HWDGE ring; output on scalar.
    - v4: mat half on sync ring + noise half on scalar ring (two HWDGE
      queues stream concurrently: HW measures 347 GB/s vs 326 single-ring);
      output via gpsimd SWDGE so it never blocks either input FIFO.
    - v4a: v4 input split but output on the scalar ring (FIFO-risk A/B).
    - gpsimd (Pool) does w = mat + noise, freeing DVE.
    - DVE does max / max_index per row segment plus one strided batch of
      index math per chunk (byte index + bit value for the packed one-hot).
    - ACT builds the bit-packed one-hot (128 B/row) as two 128-elem
      activations per segment.
    Output: packed one-hot bits, u8 [rows, j//8] (host unpackbits), so HBM
    write traffic is j/8 bytes/row instead of j.
    """
    P = 128
    f32 = mybir.dt.float32
    u32 = mybir.dt.uint32
    chunk_rows = P * r
    assert rows_per_core % chunk_rows == 0
    n_chunks = rows_per_core // chunk_rows
    ilv = out_dt == "packi"
    loadonly = mode == "v3l"
    idx_out = mode == "v6"  # u16 argmax indices, one tiny DMA per rep
    jo = j // 8

    if ilv:
        # host pre-interleaves: each chunk is one contiguous 4 MB HBM span,
        # 32 KB contiguous per partition line
        mn = nc.dram_tensor("mn", [n_chunks, P, 2, r * j], f32,
                            kind="ExternalInput")
        mnv = mn[:, :, :, :]
    else:
        mn = nc.dram_tensor("mn", [2, rows_per_core, j], f32, kind="ExternalInput")
        mnv = mn[:, :, :].rearrange("t (c p r) m -> c p t (r m)", p=P, r=r)
    if idx_out:
        # row (c*128+p)*r+s lives at out[p, c, s]; host reorders
        out = nc.dram_tensor("out", [P, n_chunks, r], mybir.dt.uint16,
                             kind="ExternalOutput")
    else:
        out = nc.dram_tensor("out", [rows_per_core, jo], mybir.dt.uint8,
                             kind="ExternalOutput")
        outv = out[:, :].rearrange("(c p r) m -> c p (r m)", p=P, r=r)

    with TileContext(nc) as tc:
        with (
            tc.tile_pool(name="const", bufs=1) as cpool,
            tc.tile_pool(name="work", bufs=bufs) as pool,
            tc.tile_pool(name="mnp", bufs=mn_bufs or bufs) as mnpool,
            tc.tile_pool(name="small", bufs=3) as spool,
        ):
            iota_i = cpool.tile([P, jo], mybir.dt.int32)
            nc.gpsimd.iota(iota_i[:], [[1, jo]], channel_multiplier=0)
            iota_f = cpool.tile([P, jo], f32)
            nc.vector.tensor_copy(iota_f[:], iota_i[:])

            split_in = mode in ("v4", "v4a")
            out_dma = nc.gpsimd if mode == "v4" else nc.scalar
            oix = None
            for c in [c for _ in range(repeat) for c in range(n_chunks)]:
                tmn = mnpool.tile([P, 2, r * j], f32, tag="mn")
                if split_in:
                    if ilv:
                        nc.sync.dma_start(tmn[:, 0, :], mnv[c, :, 0, :])
                        nc.scalar.dma_start(tmn[:, 1, :], mnv[c, :, 1, :])
                    else:
                        nc.sync.dma_start(tmn[:, 0, :], mnv[c][:, 0, :])
                        nc.scalar.dma_start(tmn[:, 1, :], mnv[c][:, 1, :])
                elif ilv:
                    nc.sync.dma_start(tmn[:, :, :], mnv[c, :, :, :])
                else:
                    nc.sync.dma_start(tmn[:, :, :], mnv[c])
                if loadonly:
                    continue
                w = pool.tile([P, r * j], f32, tag="w")
                nc.gpsimd.tensor_add(w[:], tmn[:, 0, :], tmn[:, 1, :])
                mx = spool.tile([P, 8 * r], f32, tag="mx")
                ix = spool.tile([P, 8, r], u32, tag="ix")
                for s in range(r):
                    seg = w[:, s * j : (s + 1) * j]
                    nc.vector.max(mx[:, 8 * s : 8 * s + 8], seg)
                    nc.vector.max_index(ix[:, :, s], mx[:, 8 * s : 8 * s + 8], seg)
                # batch index math over the r argmax heads (strided [P, r])
                heads = ix[:, 0, :]  # [P, r] u32, stride 8 elems
                if idx_out:
                    if c == 0:
                        oix = pool.tile([P, n_chunks * r], mybir.dt.uint16,
                                        tag="oix")
                    nc.vector.tensor_copy(oix[:, c * r : (c + 1) * r], heads)
                    if c == n_chunks - 1:
                        nc.scalar.dma_start(
                            out[:, :, :].rearrange("p c r -> p (c r)"), oix[:])
                    continue
                ot = pool.tile([P, r * jo], mybir.dt.uint8, tag="out")
                bi = spool.tile([P, r], u32, tag="bi")
                nc.vector.tensor_scalar(
                    bi[:], heads, 3, None, op0=mybir.AluOpType.logical_shift_right)
                bif = spool.tile([P, r], f32, tag="bif")
                nc.vector.tensor_scalar_mul(bif[:], bi[:], 1.0)
                rem = spool.tile([P, r], u32, tag="rem")
                nc.vector.tensor_scalar(
                    rem[:], heads, 7, None, op0=mybir.AluOpType.bitwise_and)
                vb = spool.tile([P, r], u32, tag="vb")
                nc.vector.tensor_scalar(
                    vb[:], rem[:], 127, None, op0=mybir.AluOpType.add)
                ve = spool.tile([P, r], u32, tag="ve")
                nc.vector.tensor_scalar(
                    ve[:], vb[:], 23, None, op0=mybir.AluOpType.logical_shift_left)
                nv = spool.tile([P, r], f32, tag="nv")
                nc.vector.tensor_scalar_mul(nv[:], ve[:].bitcast(f32), -1.0)
                for s in range(r):
                    oseg = ot[:, s * jo : (s + 1) * jo]
                    ab = spool.tile([P, jo], f32, tag="abs")
                    nc.scalar.activation(
                        ab[:], iota_f[:], mybir.ActivationFunctionType.Abs,
                        bias=bif[:, s : s + 1], scale=-1.0)
                    nc.scalar.activation(
                        oseg, ab[:], mybir.ActivationFunctionType.Relu,
                        bias=ve[:, s : s + 1].bitcast(f32),
                        scale=nv[:, s : s + 1])
                out_dma.dma_start(outv[c], ot[:])
    nc.finalize()
    return nc


def _build_nc_v5(nc, mybir, TileContext, rows_per_core, j, r, repeat, bufs,
                 out_dt, mn_bufs, mode="v5"):
    """v5: two-ring input streaming at full rate.

    - mat half of each chunk on the sync (SP) HWDGE ring, noise half on the
      scalar (ACT) ring.  ACT runs NO compute at all, so its ring feeds
      noise chunks back-to-back (the v4 lesson: activations in the ACT
      stream FIFO-block the next noise load).
    - output DMA rides the sync ring DELAYED BY TWO CHUNKS, so its data is
      always already computed when the trigger issues - no FIFO stall.
    - gpsimd (Pool): w = mat + noise.  DVE: max/max_index per segment,
      batched index math, and the bit-packed one-hot bytes via
      (iota == byte_idx) * 2^bit with per-partition scalar operands.
    - v5l: loads only (input-ceiling measurement variant).
    """
    P = 128
    f32 = mybir.dt.float32
    u32 = mybir.dt.uint32
    chunk_rows = P * r
    assert rows_per_core % chunk_rows == 0
    n_chunks = rows_per_core // chunk_rows
    ilv = out_dt == "packi"
    loadonly = mode == "v5l"
    no_out = mode == "v5no"     # loads + add + DVE, no output DMA
    no_add = mode == "v5na"     # loads + DVE on mat only + out, no Pool add
    one_ring = mode == "v5s2"   # split DMAs but both on sync; out on scalar
    two_tiles = mode == "v5t2"  # mat and noise in separate tiles
    jo = j // 8

    if ilv:
        mn = nc.dram_tensor("mn", [n_chunks, P, 2, r * j], f32,
                            kind="ExternalInput")
    else:
        mn = nc.dram_tensor("mn", [2, rows_per_core, j], f32, kind="ExternalInput")
        mnv = mn[:, :, :].rearrange("t (c p r) m -> c p t (r m)", p=P, r=r)
    out = nc.dram_tensor("out", [rows_per_core, jo], mybir.dt.uint8,
                         kind="ExternalOutput")
    outv = out[:, :].rearrange("(c p r) m -> c p (r m)", p=P, r=r)

    with TileContext(nc) as tc:
        with (
            tc.tile_pool(name="const", bufs=1) as cpool,
            tc.tile_pool(name="work", bufs=bufs) as pool,
            tc.tile_pool(name="mnp", bufs=mn_bufs or bufs) as mnpool,
            tc.tile_pool(name="otp", bufs=4) as opool,
            tc.tile_pool(name="small", bufs=3) as spool,
        ):
            iota_i = cpool.tile([P, jo], mybir.dt.int32)
            nc.gpsimd.iota(iota_i[:], [[1, jo]], channel_multiplier=0)
            iota_f = cpool.tile([P, jo], f32)
            nc.vector.tensor_copy(iota_f[:], iota_i[:])

            pending = []

            def flush_one():
                cc, ot_cc = pending.pop(0)
                (nc.scalar if mode == "v5s2" else nc.sync).dma_start(
                    outv[cc], ot_cc[:])

            for c in [c for _ in range(repeat) for c in range(n_chunks)]:
                noise_q = nc.sync if one_ring else nc.scalar
                if two_tiles:
                    tmat = mnpool.tile([P, r * j], f32, tag="tmat")
                    tnoi = mnpool.tile([P, r * j], f32, tag="tnoi")
                    srcm = mn[c, :, 0, :] if ilv else mnv[c][:, 0, :]
                    srcn = mn[c, :, 1, :] if ilv else mnv[c][:, 1, :]
                    nc.sync.dma_start(tmat[:], srcm)
                    noise_q.dma_start(tnoi[:], srcn)
                    t0, t1 = tmat[:], tnoi[:]
                else:
                    tmn = mnpool.tile([P, 2, r * j], f32, tag="mn")
                    if ilv:
                        nc.sync.dma_start(tmn[:, 0, :], mn[c, :, 0, :])
                        noise_q.dma_start(tmn[:, 1, :], mn[c, :, 1, :])
                    else:
                        nc.sync.dma_start(tmn[:, 0, :], mnv[c][:, 0, :])
                        noise_q.dma_start(tmn[:, 1, :], mnv[c][:, 1, :])
                    t0, t1 = tmn[:, 0, :], tmn[:, 1, :]
                if loadonly:
                    continue
                if no_add:
                    w_ap = t0
                else:
                    w = pool.tile([P, r * j], f32, tag="w")
                    nc.gpsimd.tensor_add(w[:], t0, t1)
                    w_ap = w[:]
                ot = opool.tile([P, r * jo], mybir.dt.uint8, tag="out")
                mx = spool.tile([P, 8 * r], f32, tag="mx")
                ix = spool.tile([P, 8, r], u32, tag="ix")
                for s in range(r):
                    seg = w_ap[:, s * j : (s + 1) * j]
                    nc.vector.max(mx[:, 8 * s : 8 * s + 8], seg)
                    nc.vector.max_index(ix[:, :, s], mx[:, 8 * s : 8 * s + 8], seg)
                heads = ix[:, 0, :]  # [P, r] u32, stride 8 elems
                bi = spool.tile([P, r], u32, tag="bi")
                nc.vector.tensor_scalar(
                    bi[:], heads, 3, None, op0=mybir.AluOpType.logical_shift_right)
                bif = spool.tile([P, r], f32, tag="bif")
                nc.vector.tensor_scalar_mul(bif[:], bi[:], 1.0)
                rem = spool.tile([P, r], u32, tag="rem")
                nc.vector.tensor_scalar(
                    rem[:], heads, 7, None, op0=mybir.AluOpType.bitwise_and)
                vb = spool.tile([P, r], u32, tag="vb")
                nc.vector.tensor_scalar(
                    vb[:], rem[:], 127, None, op0=mybir.AluOpType.add)
                ve = spool.tile([P, r], u32, tag="ve")
                nc.vector.tensor_scalar(
                    ve[:], vb[:], 23, None, op0=mybir.AluOpType.logical_shift_left)
                for s in range(r):
                    oseg = ot[:, s * jo : (s + 1) * jo]
                    nc.vector.tensor_scalar(
                        oseg, iota_f[:], bif[:, s : s + 1],
                        ve[:, s : s + 1].bitcast(f32),
                        op0=mybir.AluOpType.is_equal,
                        op1=mybir.AluOpType.mult)
                if no_out:
                    continue
                pending.append((c, ot))
                if len(pending) > 2:
                    if one_ring:
                        cc, ot_cc = pending.pop(0)
                        nc.scalar.dma_start(outv[cc], ot_cc[:])
                    else:
                        flush_one()
            while pending:
                flush_one()
    nc.finalize()
    return nc


def _get_nc(rows_per_core=ROWS_PER_CORE, j=J, r=4, onehot_engine=None, repeat=1,
            mode=None, bufs=2, out_engine="scalar", out_dt=None, mn_bufs=0):
    if mode is None:
        mode = os.environ.get("KERNEL_MODE", "v6")
    if onehot_engine is None:
        onehot_engine = os.environ.get("KERNEL_ONEHOT", "act")
    if out_dt is None:
        out_dt = os.environ.get("KERNEL_OUT_DT", "packi")
    key = (rows_per_core, j, r, onehot_engine, repeat, mode, bufs, out_engine, out_dt,
           mn_bufs)
    if key not in _NC_CACHE:
        _NC_CACHE[key] = _build_nc(*key)
    return _NC_CACHE[key]


def _greedy_select(w_first: np.ndarray) -> np.ndarray:
    """Sequential greedy: row r takes the available joint with max w[r].

    Equivalent to the reference's scan over descending top-k indices.
    """
    n = w_first.shape[0]
    avail = np.ones(n, dtype=bool)
    sel = np.empty(n, dtype=np.int64)
    neg_inf = np.float32(-np.inf)
    for r in range(n):
        row = np.where(avail, w_first[r], neg_inf)
        s = int(np.argmax(row))
        sel[r] = s
        avail[s] = False
    return sel


_RUNNER_CACHE = {}


def _make_runner(r: int = 4, onehot_engine=None, repeat: int = 1, mode: str = None,
                 bufs: int = 2, out_engine: str = "scalar", out_dt=None, mn_bufs: int = 0):
    """Cached runner around run_bass_kernel_spmd.

    The first call goes through run_bass_kernel_spmd (the supported axon/PJRT
    path); during it we capture the jitted SPMD callable that
    run_bass_via_pjrt builds internally, so subsequent calls (and timing
    loops) reuse the compiled executable instead of re-tracing/re-compiling
    (run_bass_via_pjrt creates a fresh jit closure per invocation).
    """
    if mode is None:
        mode = os.environ.get("KERNEL_MODE", "v6")
    if out_dt is None:
        out_dt = os.environ.get("KERNEL_OUT_DT", "packi")
    key = (r, onehot_engine, repeat, mode, bufs, out_engine, out_dt, mn_bufs)
    if key in _RUNNER_CACHE:
        return _RUNNER_CACHE[key]

    import jax
    from concourse.bass_utils import run_bass_kernel_spmd

    nc = _get_nc(ROWS_PER_CORE, J, r, onehot_engine, repeat, mode, bufs, out_engine,
                 out_dt, mn_bufs)
    state = {"fn": None}

    def runner(mn_global: np.ndarray) -> np.ndarray:
        """mn_global: (2*N_CORES, ROWS_PER_CORE, J) per-core [mat, noise]
        pairs. Returns (HW, J) output."""
        if state["fn"] is None:
            per = mn_global.shape[0] // N_CORES
            in_maps = [{"mn": mn_global[per * k : per * (k + 1)]} for k in range(N_CORES)]
            orig_jit = jax.jit

            def capturing_jit(f, *a, **kw):
                g = orig_jit(f, *a, **kw)
                if "donate_argnums" in kw and kw.get("keep_unused"):
                    state["fn"] = g
                return g

            jax.jit = capturing_jit
            try:
                res = run_bass_kernel_spmd(nc, in_maps, core_ids=list(range(N_CORES)))
            finally:
                jax.jit = orig_jit
            out = np.concatenate([r_["out"] for r_ in res.results], axis=0)
            state["out_np_dtype"] = out.dtype
            state["out_shape"] = out.shape
            return out
        outs = state["fn"](mn_global, np.zeros(state["out_shape"], state["out_np_dtype"]))
        out = outs[0] if isinstance(outs, (tuple, list)) else outs
        return np.asarray(out)

    runner.state = state
    runner.stack = ((lambda m, n: stack_inputs_ilv(m, n, r))
                    if out_dt == "packi" else stack_inputs)
    _RUNNER_CACHE[key] = runner
    return runner


def stack_inputs(mat: np.ndarray, noise: np.ndarray) -> np.ndarray:
    """Global (2*N_CORES, ROWS_PER_CORE, J): per-core [mat_shard, noise_shard]
    pairs along axis 0, so a P("core") shard is exactly the NEFF's (2, rows, J)
    "mn" tensor."""
    m3 = mat.reshape(N_CORES, ROWS_PER_CORE, J)
    n3 = noise.reshape(N_CORES, ROWS_PER_CORE, J)
    return np.stack([m3, n3], axis=1).reshape(2 * N_CORES, ROWS_PER_CORE, J)


def stack_inputs_ilv(mat: np.ndarray, noise: np.ndarray, r: int = 4) -> np.ndarray:
    """Interleaved layout: global (N_CORES*n_chunks, P, 2, r*J); every chunk is
    one contiguous 4 MB block on device."""
    nck = ROWS_PER_CORE // (P * r)
    m5 = mat.reshape(N_CORES * nck, P, r * J)
    n5 = noise.reshape(N_CORES * nck, P, r * J)
    return np.ascontiguousarray(np.stack([m5, n5], axis=2))


def run_device(mat: np.ndarray, noise: np.ndarray, r: int = 4, onehot_engine=None):
    """Shard row-wise over 8 cores, run the Bass kernel, gather."""
    runner = _make_runner(r, onehot_engine)
    out = runner(runner.stack(mat, noise))
    return np.asarray(out)


def kernel(sgt_trans_mat, gumbel_noise, use_gumbel_noise=1, is_training=1,
           temperature=30):
    mat = np.ascontiguousarray(np.asarray(sgt_trans_mat, dtype=np.float32))
    assert mat.shape == (HW, J), mat.shape
    training = bool(int(np.asarray(is_training)))
    use_g = training and bool(int(np.asarray(use_gumbel_noise)))
    if use_g:
        noise = np.ascontiguousarray(np.asarray(gumbel_noise, dtype=np.float32))
    else:
        # selection order falls back to mat itself; temperature never matters
        noise = np.zeros_like(mat)

    out = run_device(mat, noise)
    # device output may be u16 indices / bit-packed / uint8 / bf16; f32 it
    if out.ndim == 3:  # u16 argmax indices: [N_CORES*128, n_chunks, r]
        n_chunks, r = out.shape[1], out.shape[2]
        sel_dev = (out.astype(np.int64)
                   .reshape(N_CORES, P, n_chunks, r)
                   .transpose(0, 2, 1, 3)  # row (c*128+p)*r+s order
                   .reshape(HW))
        out = np.zeros((HW, J), np.float32)
        out[np.arange(HW), sel_dev] = np.float32(1.0)
    elif out.shape[1] == J // 8:
        out = np.unpackbits(np.ascontiguousarray(out), axis=1,
                            bitorder="little").astype(np.float32)
    elif out.dtype != np.float32:
        out = out.astype(np.float32)
    elif not out.flags.writeable:
        out = out.copy()

    # Host-side greedy over the first J rows (inherently sequential, tiny),
    # then patch those rows of the output.
    w_first = mat[:J] + noise[:J]  # same IEEE fp32 add as the device
    sel = _greedy_select(w_first)
    out[:J] = 0.0
    out[np.arange(J), sel] = np.float32(1.0)
    return out



# revision 36
# speedup vs baseline: 1.0554x; 1.0154x over previous
"""Trainium2 Bass kernel for nn_AutoEncIndex_33887291965861 (topk_masking).

Reference computation:
    soft  = softmax((mat + noise) / temperature)            [training w/ gumbel]
    index = top_k(soft, J).indices                          (full descending sort)
    sel   = greedy row-by-row assignment (first J rows pick the best
            still-unused joint; later rows pick their argmax)
    out   = stop_grad(one_hot(sel)) - stop_grad(mat) + mat

Key facts used here:
  * (0 - m) + m == +0.0 exactly in IEEE fp32, so the output is an exact
    one-hot matrix except the selected entry is (1 - m) + m which is within
    1-2 ulp of 1.0.  Emitting exactly 1.0 keeps the total relative error
    at ~2e-7.
  * softmax and /temperature are strictly monotone per row, so the selection
    order is the order of w = mat + noise (fp32), with lowest-index
    tie-breaking (lax.top_k semantics == vector-engine max_index semantics).
  * The greedy pass over the first J rows selects, for row r, the
    still-available joint with the highest w[r] value (proof: the first
    available joint in row r's descending order always sits within the
    first r+1 positions by pigeonhole, which is exactly the cols<=r window
    the reference uses).  Rows >= J just take their argmax.

Device kernel (SPMD over 8 cores, row-sharded, 4096 rows/core; mode "v6"
with out_dt "packi" - the measured-fastest configuration; "v6" replaces
the bit-packed one-hot output with u16 argmax indices, one 64 KB DMA per
pass, host scatters the exact one-hot - removes the 8 per-chunk output
DMA insertions and all ACT work; T = 97.2 us vs 100.2 for packed bits vs
94.4 us loads-only floor, same-session HW comparison.  The "v3" text
below describes the shared input/compute pipeline):
  * Input: host interleaves [mat;noise] so every 4 MB chunk (512 rows) is
    one fully-contiguous HBM span, 32 KB contiguous per partition line.
    A single chunk DMA on the sync (SP) HWDGE ring, and that ring carries
    ONLY input loads, so chunks stream back-to-back at the measured
    ~345 GB/s per-core rate (the split mat/noise descriptor layout only
    reaches ~326 GB/s; splitting input across both HWDGE rings reaches
    347 GB/s loads-only but collapses to ~140 us/pass when compute
    coexists, so single-ring contiguous wins).
  * w = mat + noise on gpsimd (Pool), freeing the vector engine.
  * DVE: per-segment argmax via max/max_index, plus one strided batch of
    index math per chunk (byte index bi = idx>>3, bit value 2^(idx&7) as
    f32 bits) feeding the packed one-hot.
  * ACT: bit-packed one-hot (128 B/row): ab = Abs(-iota128 + bi), then
    byte = Relu(ab*(-v) + v) = v at the byte position, 0 elsewhere
    (v = 2^(idx&7), exact in u8).  Output DMA on the ACT HWDGE ring so it
    never stalls the input FIFO.
  Memory bound: 32.5 MB HBM traffic per core per pass (32 MiB in +
  512 KB packed out); steady state ~98 us/pass = within ~2% of the
  measured pure-input-DMA ceiling (~96.6 us).  The vector engine
  (~9.5 us/chunk), Pool (~8.5 us/chunk) and ACT (~3 us/chunk) all sit
  below the 12.2 us/chunk DMA period, fully overlapped.

Host: the inherently-sequential greedy over the first 1024 rows (tiny), then
patch those rows of the gathered output; unpackbits decodes the device's
bit-packed one-hot (exact).
"""

import os

import numpy as np

HW = 32768
J = 1024
N_CORES = 8
ROWS_PER_CORE = HW // N_CORES  # 4096
P = 128  # SBUF partitions

_NC_CACHE = {}


def _build_nc(rows_per_core: int, j: int, r: int, onehot_engine: str = "act",
              repeat: int = 1, mode: str = "full", bufs: int = 2,
              out_engine: str = "sync", out_dt: str = "u8", mn_bufs: int = 0):
    """Build the per-core Bass module.

    Input "mn" is [2, rows_per_core, j] fp32 — mat stacked with noise (one
    tensor so each chunk loads with a single DMA instruction / single
    semaphore: TRN2 compute instructions can carry only one sync wait).
    Output "out" is the exact one-hot of the per-row argmax of mat + noise.
    r = rows per partition per chunk (chunk covers 128*r rows).
    """
    import concourse.bacc as bacc
    import concourse.mybir as mybir
    from concourse.tile import TileContext

    chunk_rows = P * r
    assert rows_per_core % chunk_rows == 0, (rows_per_core, chunk_rows)
    n_chunks = rows_per_core // chunk_rows
    f32 = mybir.dt.float32

    # Bacc (not raw Bass): its finalize() runs generate_event_semaphores,
    # which splits multi-sem waits — TRN2 instructions carry at most one.
    nc = bacc.Bacc()
    if mode in ("v3", "v3l", "v4", "v4a", "v6"):
        return _build_nc_v3(nc, mybir, TileContext, rows_per_core, j, r, repeat,
                            bufs, out_dt, mn_bufs, mode)
    if mode.startswith("v5"):
        return _build_nc_v5(nc, mybir, TileContext, rows_per_core, j, r, repeat,
                            bufs, out_dt, mn_bufs, mode)
    pack = out_dt == "pack"
    odt = {"f32": f32, "u8": mybir.dt.uint8, "bf16": mybir.dt.bfloat16,
           "pack": mybir.dt.uint8}[out_dt]
    # packed mode: 1024 one-hot bits -> 128 bytes per row (byte idx>>3 holds
    # 1 << (idx & 7)); host unpacks with np.unpackbits(bitorder="little")
    jo = j // 8 if pack else j
    ilv = mode == "ilv"
    if ilv:
        # host pre-interleaves so every chunk DMA reads one fully-contiguous
        # 4 MB block (single HBM stream instead of mat/noise 16 MB apart)
        mn = nc.dram_tensor(
            "mn", [rows_per_core // (P * r), P, 2, r * j], f32,
            kind="ExternalInput")
        mnv = mn[:, :, :, :]
    else:
        mn = nc.dram_tensor("mn", [2, rows_per_core, j], f32, kind="ExternalInput")
        # chunk c, partition p holds rows (c*128 + p)*r .. +r-1
        mnv = mn[:, :, :].rearrange("t (c p r) m -> c p t (r m)", p=P, r=r)
    out = nc.dram_tensor("out", [rows_per_core, jo], odt, kind="ExternalOutput")
    outv = out[:, :].rearrange("(c p r) m -> c p (r m)", p=P, r=r)

    out_dma = {"sync": nc.sync, "scalar": nc.scalar, "gpsimd": nc.gpsimd}[out_engine]
    with TileContext(nc) as tc:
        with (
            tc.tile_pool(name="const", bufs=1) as cpool,
            tc.tile_pool(name="work", bufs=bufs) as pool,
            tc.tile_pool(name="mnp", bufs=mn_bufs or bufs) as mnpool,
            tc.tile_pool(name="small", bufs=3) as spool,
        ):
            iota_i = cpool.tile([P, j], mybir.dt.int32)
            nc.gpsimd.iota(iota_i[:], [[1, j]], channel_multiplier=0)
            iota_f = cpool.tile([P, j], f32)
            nc.vector.tensor_copy(iota_f[:], iota_i[:])

            for c in [c for _ in range(repeat) for c in range(n_chunks)]:
                tmn = mnpool.tile([P, 2, r * j], f32, tag="mn")
                if mode in ("split2", "loadonly2"):
                    # mat half on the SP HWDGE ring, noise half on the ACT ring
                    nc.sync.dma_start(tmn[:, 0, :], mnv[c][:, 0, :])
                    nc.scalar.dma_start(tmn[:, 1, :], mnv[c][:, 1, :])
                else:
                    nc.sync.dma_start(tmn[:, :, :], mnv[c])
                if mode in ("loadonly", "loadonly2"):
                    continue
                if mode == "dmaonly":
                    ot = pool.tile([P, r * j], odt, tag="out")
                    nc.vector.tensor_copy(ot[:], tmn[:, 0, :])
                    out_dma.dma_start(outv[c], ot[:])
                    continue
                w = pool.tile([P, r * j], f32, tag="w")
                nc.vector.tensor_add(w[:], tmn[:, 0, :], tmn[:, 1, :])
                ot = pool.tile([P, r * jo], odt, tag="out")
                mx = spool.tile([P, 8 * r], f32, tag="mx")
                ix = spool.tile([P, 8 * r], mybir.dt.uint32, tag="ix")
                for s in range(r):
                    seg = w[:, s * j : (s + 1) * j]
                    oseg = ot[:, s * jo : (s + 1) * jo]
                    nc.vector.max(mx[:, 8 * s : 8 * s + 8], seg)
                    nc.vector.max_index(ix[:, 8 * s : 8 * s + 8], mx[:, 8 * s : 8 * s + 8], seg)
                    if pack:
                        ixs = ix[:, 8 * s : 8 * s + 1]
                        bi = spool.tile([P, 1], mybir.dt.uint32, tag="bi")
                        nc.vector.tensor_scalar(
                            bi[:], ixs, 3, None, op0=mybir.AluOpType.logical_shift_right)
                        rem = spool.tile([P, 1], mybir.dt.uint32, tag="rem")
                        nc.vector.tensor_scalar(
                            rem[:], ixs, 7, None, op0=mybir.AluOpType.bitwise_and)
                        # v = 2^rem exactly: f32 bit pattern (rem+127) << 23
                        vb = spool.tile([P, 1], mybir.dt.uint32, tag="vb")
                        nc.vector.tensor_scalar(
                            vb[:], rem[:], 127, None, op0=mybir.AluOpType.add)
                        ve = spool.tile([P, 1], mybir.dt.uint32, tag="ve")
                        nc.vector.tensor_scalar(
                            ve[:], vb[:], 23, None,
                            op0=mybir.AluOpType.logical_shift_left)
                        bf = spool.tile([P, 1], f32, tag="bf")
                        nc.vector.tensor_scalar_mul(bf[:], bi[:], 1.0)
                        nc.vector.tensor_scalar(
                            oseg, iota_f[:, :jo], bf[:], ve[:].bitcast(f32),
                            op0=mybir.AluOpType.is_equal,
                            op1=mybir.AluOpType.mult)
                    elif onehot_engine == "act":
                        # one-hot on the scalar engine: Relu(1 - |iota - idx|)
                        ixn = spool.tile([P, 1], f32, tag="ixn")
                        nc.vector.tensor_scalar_mul(ixn[:], ix[:, 8 * s : 8 * s + 1], -1.0)
                        ab = spool.tile([P, j], f32, tag="abs")
                        nc.scalar.activation(
                            ab[:], iota_f[:], mybir.ActivationFunctionType.Abs,
                            bias=ixn[:], scale=1.0,
                        )
                        nc.scalar.activation(
                            oseg, ab[:], mybir.ActivationFunctionType.Relu,
                            bias=1.0, scale=-1.0,
                        )
                    else:
                        # one-hot on the vector engine: (iota == idx), f32 compare
                        ixf = spool.tile([P, 1], f32, tag="ixf")
                        nc.vector.tensor_scalar_mul(ixf[:], ix[:, 8 * s : 8 * s + 1], 1.0)
                        nc.vector.tensor_scalar(
                            oseg, iota_f[:], ixf[:], None,
                            op0=mybir.AluOpType.is_equal,
                        )
                out_dma.dma_start(outv[c], ot[:])
    nc.finalize()
    return nc


def _build_nc_v3(nc, mybir, TileContext, rows_per_core, j, r, repeat, bufs,
                 out_dt, mn_bufs, mode="v3"):
    """v3/v4: engine-balanced, DMA-roofline layout.

    - v3: one 4 MB chunk DMA on the sync (SP) HWDGE ring; output on scalar.
    - v4: mat half on sync ring + noise half on scalar ring (two HWDGE
      queues stream concurrently: HW measures 347 GB/s vs 326 single-ring);
      output via gpsimd SWDGE so it never blocks either input FIFO.
    - v4a: v4 input split but output on the scalar ring (FIFO-risk A/B).
    - gpsimd (Pool) does w = mat + noise, freeing DVE.
    - DVE does max / max_index per row segment plus one strided batch of
      index math per chunk (byte index + bit value for the packed one-hot).
    - ACT builds the bit-packed one-hot (128 B/row) as two 128-elem
      activations per segment.
    Output: packed one-hot bits, u8 [rows, j//8] (host unpackbits), so HBM
    write traffic is j/8 bytes/row instead of j.
    """
    P = 128
    f32 = mybir.dt.float32
    u32 = mybir.dt.uint32
    chunk_rows = P * r
    assert rows_per_core % chunk_rows == 0
    n_chunks = rows_per_core // chunk_rows
    ilv = out_dt == "packi"
    loadonly = mode == "v3l"
    idx_out = mode == "v6"  # u16 argmax indices, one tiny DMA per rep
    jo = j // 8

    if ilv:
        # host pre-interleaves: each chunk is one contiguous 4 MB HBM span,
        # 32 KB contiguous per partition line
        mn = nc.dram_tensor("mn", [n_chunks, P, 2, r * j], f32,
                            kind="ExternalInput")
        mnv = mn[:, :, :, :]
    else:
        mn = nc.dram_tensor("mn", [2, rows_per_core, j], f32, kind="ExternalInput")
        mnv = mn[:, :, :].rearrange("t (c p r) m -> c p t (r m)", p=P, r=r)
    if idx_out:
        # row (c*128+p)*r+s lives at out[p, c, s]; host reorders
        out = nc.dram_tensor("out", [P, n_chunks, r], mybir.dt.uint16,
                             kind="ExternalOutput")
    else:
        out = nc.dram_tensor("out", [rows_per_core, jo], mybir.dt.uint8,
                             kind="ExternalOutput")
        outv = out[:, :].rearrange("(c p r) m -> c p (r m)", p=P, r=r)

    with TileContext(nc) as tc:
        with (
            tc.tile_pool(name="const", bufs=1) as cpool,
            tc.tile_pool(name="work", bufs=bufs) as pool,
            tc.tile_pool(name="mnp", bufs=mn_bufs or bufs) as mnpool,
            tc.tile_pool(name="small", bufs=3) as spool,
        ):
            iota_i = cpool.tile([P, jo], mybir.dt.int32)
            nc.gpsimd.iota(iota_i[:], [[1, jo]], channel_multiplier=0)
            iota_f = cpool.tile([P, jo], f32)
            nc.vector.tensor_copy(iota_f[:], iota_i[:])

            split_in = mode in ("v4", "v4a")
            out_dma = nc.gpsimd if mode == "v4" else nc.scalar
            oix = None
            for c in [c for _ in range(repeat) for c in range(n_chunks)]:
                tmn = mnpool.tile([P, 2, r * j], f32, tag="mn")
                if split_in:
                    if ilv:
                        nc.sync.dma_start(tmn[:, 0, :], mnv[c, :, 0, :])
                        nc.scalar.dma_start(tmn[:, 1, :], mnv[c, :, 1, :])
                    else:
                        nc.sync.dma_start(tmn[:, 0, :], mnv[c][:, 0, :])
                        nc.scalar.dma_start(tmn[:, 1, :], mnv[c][:, 1, :])
                elif ilv:
                    nc.sync.dma_start(tmn[:, :, :], mnv[c, :, :, :])
                else:
                    nc.sync.dma_start(tmn[:, :, :], mnv[c])
                if loadonly:
                    continue
                w = pool.tile([P, r * j], f32, tag="w")
                nc.gpsimd.tensor_add(w[:], tmn[:, 0, :], tmn[:, 1, :])
                mx = spool.tile([P, 8 * r], f32, tag="mx")
                ix = spool.tile([P, 8, r], u32, tag="ix")
                for s in range(r):
                    seg = w[:, s * j : (s + 1) * j]
                    nc.vector.max(mx[:, 8 * s : 8 * s + 8], seg)
                    nc.vector.max_index(ix[:, :, s], mx[:, 8 * s : 8 * s + 8], seg)
                # batch index math over the r argmax heads (strided [P, r])
                heads = ix[:, 0, :]  # [P, r] u32, stride 8 elems
                if idx_out:
                    if c == 0:
                        oix = pool.tile([P, n_chunks * r], mybir.dt.uint16,
                                        tag="oix")
                    nc.vector.tensor_copy(oix[:, c * r : (c + 1) * r], heads)
                    if c == n_chunks - 1:
                        nc.scalar.dma_start(
                            out[:, :, :].rearrange("p c r -> p (c r)"), oix[:])
                    continue
                ot = pool.tile([P, r * jo], mybir.dt.uint8, tag="out")
                bi = spool.tile([P, r], u32, tag="bi")
                nc.vector.tensor_scalar(
                    bi[:], heads, 3, None, op0=mybir.AluOpType.logical_shift_right)
                bif = spool.tile([P, r], f32, tag="bif")
                nc.vector.tensor_scalar_mul(bif[:], bi[:], 1.0)
                rem = spool.tile([P, r], u32, tag="rem")
                nc.vector.tensor_scalar(
                    rem[:], heads, 7, None, op0=mybir.AluOpType.bitwise_and)
                vb = spool.tile([P, r], u32, tag="vb")
                nc.vector.tensor_scalar(
                    vb[:], rem[:], 127, None, op0=mybir.AluOpType.add)
                ve = spool.tile([P, r], u32, tag="ve")
                nc.vector.tensor_scalar(
                    ve[:], vb[:], 23, None, op0=mybir.AluOpType.logical_shift_left)
                nv = spool.tile([P, r], f32, tag="nv")
                nc.vector.tensor_scalar_mul(nv[:], ve[:].bitcast(f32), -1.0)
                for s in range(r):
                    oseg = ot[:, s * jo : (s + 1) * jo]
                    ab = spool.tile([P, jo], f32, tag="abs")
                    nc.scalar.activation(
                        ab[:], iota_f[:], mybir.ActivationFunctionType.Abs,
                        bias=bif[:, s : s + 1], scale=-1.0)
                    nc.scalar.activation(
                        oseg, ab[:], mybir.ActivationFunctionType.Relu,
                        bias=ve[:, s : s + 1].bitcast(f32),
                        scale=nv[:, s : s + 1])
                out_dma.dma_start(outv[c], ot[:])
    nc.finalize()
    return nc


def _build_nc_v5(nc, mybir, TileContext, rows_per_core, j, r, repeat, bufs,
                 out_dt, mn_bufs, mode="v5"):
    """v5: two-ring input streaming at full rate.

    - mat half of each chunk on the sync (SP) HWDGE ring, noise half on the
      scalar (ACT) ring.  ACT runs NO compute at all, so its ring feeds
      noise chunks back-to-back (the v4 lesson: activations in the ACT
      stream FIFO-block the next noise load).
    - output DMA rides the sync ring DELAYED BY TWO CHUNKS, so its data is
      always already computed when the trigger issues - no FIFO stall.
    - gpsimd (Pool): w = mat + noise.  DVE: max/max_index per segment,
      batched index math, and the bit-packed one-hot bytes via
      (iota == byte_idx) * 2^bit with per-partition scalar operands.
    - v5l: loads only (input-ceiling measurement variant).
    """
    P = 128
    f32 = mybir.dt.float32
    u32 = mybir.dt.uint32
    chunk_rows = P * r
    assert rows_per_core % chunk_rows == 0
    n_chunks = rows_per_core // chunk_rows
    ilv = out_dt == "packi"
    loadonly = mode == "v5l"
    no_out = mode == "v5no"     # loads + add + DVE, no output DMA
    no_add = mode == "v5na"     # loads + DVE on mat only + out, no Pool add
    one_ring = mode == "v5s2"   # split DMAs but both on sync; out on scalar
    two_tiles = mode == "v5t2"  # mat and noise in separate tiles
    jo = j // 8

    if ilv:
        mn = nc.dram_tensor("mn", [n_chunks, P, 2, r * j], f32,
                            kind="ExternalInput")
    else:
        mn = nc.dram_tensor("mn", [2, rows_per_core, j], f32, kind="ExternalInput")
        mnv = mn[:, :, :].rearrange("t (c p r) m -> c p t (r m)", p=P, r=r)
    out = nc.dram_tensor("out", [rows_per_core, jo], mybir.dt.uint8,
                         kind="ExternalOutput")
    outv = out[:, :].rearrange("(c p r) m -> c p (r m)", p=P, r=r)

    with TileContext(nc) as tc:
        with (
            tc.tile_pool(name="const", bufs=1) as cpool,
            tc.tile_pool(name="work", bufs=bufs) as pool,
            tc.tile_pool(name="mnp", bufs=mn_bufs or bufs) as mnpool,
            tc.tile_pool(name="otp", bufs=4) as opool,
            tc.tile_pool(name="small", bufs=3) as spool,
        ):
            iota_i = cpool.tile([P, jo], mybir.dt.int32)
            nc.gpsimd.iota(iota_i[:], [[1, jo]], channel_multiplier=0)
            iota_f = cpool.tile([P, jo], f32)
            nc.vector.tensor_copy(iota_f[:], iota_i[:])

            pending = []

            def flush_one():
                cc, ot_cc = pending.pop(0)
                (nc.scalar if mode == "v5s2" else nc.sync).dma_start(
                    outv[cc], ot_cc[:])

            for c in [c for _ in range(repeat) for c in range(n_chunks)]:
                noise_q = nc.sync if one_ring else nc.scalar
                if two_tiles:
                    tmat = mnpool.tile([P, r * j], f32, tag="tmat")
                    tnoi = mnpool.tile([P, r * j], f32, tag="tnoi")
                    srcm = mn[c, :, 0, :] if ilv else mnv[c][:, 0, :]
                    srcn = mn[c, :, 1, :] if ilv else mnv[c][:, 1, :]
                    nc.sync.dma_start(tmat[:], srcm)
                    noise_q.dma_start(tnoi[:], srcn)
                    t0, t1 = tmat[:], tnoi[:]
                else:
                    tmn = mnpool.tile([P, 2, r * j], f32, tag="mn")
                    if ilv:
                        nc.sync.dma_start(tmn[:, 0, :], mn[c, :, 0, :])
                        noise_q.dma_start(tmn[:, 1, :], mn[c, :, 1, :])
                    else:
                        nc.sync.dma_start(tmn[:, 0, :], mnv[c][:, 0, :])
                        noise_q.dma_start(tmn[:, 1, :], mnv[c][:, 1, :])
                    t0, t1 = tmn[:, 0, :], tmn[:, 1, :]
                if loadonly:
                    continue
                if no_add:
                    w_ap = t0
                else:
                    w = pool.tile([P, r * j], f32, tag="w")
                    nc.gpsimd.tensor_add(w[:], t0, t1)
                    w_ap = w[:]
                ot = opool.tile([P, r * jo], mybir.dt.uint8, tag="out")
                mx = spool.tile([P, 8 * r], f32, tag="mx")
                ix = spool.tile([P, 8, r], u32, tag="ix")
                for s in range(r):
                    seg = w_ap[:, s * j : (s + 1) * j]
                    nc.vector.max(mx[:, 8 * s : 8 * s + 8], seg)
                    nc.vector.max_index(ix[:, :, s], mx[:, 8 * s : 8 * s + 8], seg)
                heads = ix[:, 0, :]  # [P, r] u32, stride 8 elems
                bi = spool.tile([P, r], u32, tag="bi")
                nc.vector.tensor_scalar(
                    bi[:], heads, 3, None, op0=mybir.AluOpType.logical_shift_right)
                bif = spool.tile([P, r], f32, tag="bif")
                nc.vector.tensor_scalar_mul(bif[:], bi[:], 1.0)
                rem = spool.tile([P, r], u32, tag="rem")
                nc.vector.tensor_scalar(
                    rem[:], heads, 7, None, op0=mybir.AluOpType.bitwise_and)
                vb = spool.tile([P, r], u32, tag="vb")
                nc.vector.tensor_scalar(
                    vb[:], rem[:], 127, None, op0=mybir.AluOpType.add)
                ve = spool.tile([P, r], u32, tag="ve")
                nc.vector.tensor_scalar(
                    ve[:], vb[:], 23, None, op0=mybir.AluOpType.logical_shift_left)
                for s in range(r):
                    oseg = ot[:, s * jo : (s + 1) * jo]
                    nc.vector.tensor_scalar(
                        oseg, iota_f[:], bif[:, s : s + 1],
                        ve[:, s : s + 1].bitcast(f32),
                        op0=mybir.AluOpType.is_equal,
                        op1=mybir.AluOpType.mult)
                if no_out:
                    continue
                pending.append((c, ot))
                if len(pending) > 2:
                    if one_ring:
                        cc, ot_cc = pending.pop(0)
                        nc.scalar.dma_start(outv[cc], ot_cc[:])
                    else:
                        flush_one()
            while pending:
                flush_one()
    nc.finalize()
    return nc


def _get_nc(rows_per_core=ROWS_PER_CORE, j=J, r=2, onehot_engine=None, repeat=1,
            mode=None, bufs=4, out_engine="scalar", out_dt=None, mn_bufs=0):
    if mode is None:
        mode = os.environ.get("KERNEL_MODE", "v6")
    if onehot_engine is None:
        onehot_engine = os.environ.get("KERNEL_ONEHOT", "act")
    if out_dt is None:
        out_dt = os.environ.get("KERNEL_OUT_DT", "packi")
    key = (rows_per_core, j, r, onehot_engine, repeat, mode, bufs, out_engine, out_dt,
           mn_bufs)
    if key not in _NC_CACHE:
        _NC_CACHE[key] = _build_nc(*key)
    return _NC_CACHE[key]


def _greedy_select(w_first: np.ndarray) -> np.ndarray:
    """Sequential greedy: row r takes the available joint with max w[r].

    Equivalent to the reference's scan over descending top-k indices.
    """
    n = w_first.shape[0]
    avail = np.ones(n, dtype=bool)
    sel = np.empty(n, dtype=np.int64)
    neg_inf = np.float32(-np.inf)
    for r in range(n):
        row = np.where(avail, w_first[r], neg_inf)
        s = int(np.argmax(row))
        sel[r] = s
        avail[s] = False
    return sel


_RUNNER_CACHE = {}


def _make_runner(r: int = 2, onehot_engine=None, repeat: int = 1, mode: str = None,
                 bufs: int = 4, out_engine: str = "scalar", out_dt=None, mn_bufs: int = 0):
    """Cached runner around run_bass_kernel_spmd.

    The first call goes through run_bass_kernel_spmd (the supported axon/PJRT
    path); during it we capture the jitted SPMD callable that
    run_bass_via_pjrt builds internally, so subsequent calls (and timing
    loops) reuse the compiled executable instead of re-tracing/re-compiling
    (run_bass_via_pjrt creates a fresh jit closure per invocation).
    """
    if mode is None:
        mode = os.environ.get("KERNEL_MODE", "v6")
    if out_dt is None:
        out_dt = os.environ.get("KERNEL_OUT_DT", "packi")
    key = (r, onehot_engine, repeat, mode, bufs, out_engine, out_dt, mn_bufs)
    if key in _RUNNER_CACHE:
        return _RUNNER_CACHE[key]

    import jax
    from concourse.bass_utils import run_bass_kernel_spmd

    nc = _get_nc(ROWS_PER_CORE, J, r, onehot_engine, repeat, mode, bufs, out_engine,
                 out_dt, mn_bufs)
    state = {"fn": None}

    def runner(mn_global: np.ndarray) -> np.ndarray:
        """mn_global: (2*N_CORES, ROWS_PER_CORE, J) per-core [mat, noise]
        pairs. Returns (HW, J) output."""
        if state["fn"] is None:
            per = mn_global.shape[0] // N_CORES
            in_maps = [{"mn": mn_global[per * k : per * (k + 1)]} for k in range(N_CORES)]
            orig_jit = jax.jit

            def capturing_jit(f, *a, **kw):
                g = orig_jit(f, *a, **kw)
                if "donate_argnums" in kw and kw.get("keep_unused"):
                    state["fn"] = g
                return g

            jax.jit = capturing_jit
            try:
                res = run_bass_kernel_spmd(nc, in_maps, core_ids=list(range(N_CORES)))
            finally:
                jax.jit = orig_jit
            out = np.concatenate([r_["out"] for r_ in res.results], axis=0)
            state["out_np_dtype"] = out.dtype
            state["out_shape"] = out.shape
            return out
        outs = state["fn"](mn_global, np.zeros(state["out_shape"], state["out_np_dtype"]))
        out = outs[0] if isinstance(outs, (tuple, list)) else outs
        return np.asarray(out)

    runner.state = state
    runner.stack = ((lambda m, n: stack_inputs_ilv(m, n, r))
                    if out_dt == "packi" else stack_inputs)
    _RUNNER_CACHE[key] = runner
    return runner


def stack_inputs(mat: np.ndarray, noise: np.ndarray) -> np.ndarray:
    """Global (2*N_CORES, ROWS_PER_CORE, J): per-core [mat_shard, noise_shard]
    pairs along axis 0, so a P("core") shard is exactly the NEFF's (2, rows, J)
    "mn" tensor."""
    m3 = mat.reshape(N_CORES, ROWS_PER_CORE, J)
    n3 = noise.reshape(N_CORES, ROWS_PER_CORE, J)
    return np.stack([m3, n3], axis=1).reshape(2 * N_CORES, ROWS_PER_CORE, J)


def stack_inputs_ilv(mat: np.ndarray, noise: np.ndarray, r: int = 2) -> np.ndarray:
    """Interleaved layout: global (N_CORES*n_chunks, P, 2, r*J); every chunk is
    one contiguous 4 MB block on device."""
    nck = ROWS_PER_CORE // (P * r)
    m5 = mat.reshape(N_CORES * nck, P, r * J)
    n5 = noise.reshape(N_CORES * nck, P, r * J)
    return np.ascontiguousarray(np.stack([m5, n5], axis=2))


def run_device(mat: np.ndarray, noise: np.ndarray, r: int = 2, onehot_engine=None):
    """Shard row-wise over 8 cores, run the Bass kernel, gather."""
    runner = _make_runner(r, onehot_engine)
    out = runner(runner.stack(mat, noise))
    return np.asarray(out)


def kernel(sgt_trans_mat, gumbel_noise, use_gumbel_noise=1, is_training=1,
           temperature=30):
    mat = np.ascontiguousarray(np.asarray(sgt_trans_mat, dtype=np.float32))
    assert mat.shape == (HW, J), mat.shape
    training = bool(int(np.asarray(is_training)))
    use_g = training and bool(int(np.asarray(use_gumbel_noise)))
    if use_g:
        noise = np.ascontiguousarray(np.asarray(gumbel_noise, dtype=np.float32))
    else:
        # selection order falls back to mat itself; temperature never matters
        noise = np.zeros_like(mat)

    out = run_device(mat, noise)
    # device output may be u16 indices / bit-packed / uint8 / bf16; f32 it
    if out.ndim == 3:  # u16 argmax indices: [N_CORES*128, n_chunks, r]
        n_chunks, r = out.shape[1], out.shape[2]
        sel_dev = (out.astype(np.int64)
                   .reshape(N_CORES, P, n_chunks, r)
                   .transpose(0, 2, 1, 3)  # row (c*128+p)*r+s order
                   .reshape(HW))
        out = np.zeros((HW, J), np.float32)
        out[np.arange(HW), sel_dev] = np.float32(1.0)
    elif out.shape[1] == J // 8:
        out = np.unpackbits(np.ascontiguousarray(out), axis=1,
                            bitorder="little").astype(np.float32)
    elif out.dtype != np.float32:
        out = out.astype(np.float32)
    elif not out.flags.writeable:
        out = out.copy()

    # Host-side greedy over the first J rows (inherently sequential, tiny),
    # then patch those rows of the output.
    w_first = mat[:J] + noise[:J]  # same IEEE fp32 add as the device
    sel = _greedy_select(w_first)
    out[:J] = 0.0
    out[np.arange(J), sel] = np.float32(1.0)
    return out

